# revision 15
# baseline (speedup 1.0000x reference)
"""Trainium2 Bass kernel for nn_CA_Module (channel-attention + SE gating).

Reference computation per sample (C=512, N=H*W=4096):
    q = x.reshape(C, N)
    energy = q @ q.T                     # [C, C]
    att = softmax(max_row - energy)      # == softmax(-energy)  (row shift cancels)
        -> G = exp(min_row - energy); att = G / rowsum(G)
    out = att @ q                        # [C, N]
    pooled = concat([mean_n(x), mean_n(out)])        # [2C]
    h  = relu(w1 @ pooled + b1)                      # [64]
    se = sigmoid(w2 @ h + b2)                        # [C]
    y  = se * x + (1 - se) * out

Key algebraic tricks used here:
  * softmax(max-e) == softmax(-e): compute G = exp(min_row - e) directly.
  * energy is symmetric, so G^T (needed as the stationary operand of the
    second matmul) is obtained by 16 cheap PE tile-transposes of G.
  * out = diag(1/S) (G @ q), so normalization folds into the final blend:
        y = se*x + beta*(G@q),  beta = (1-se)/S
  * mean_n(out) = G @ mean_n(x) / S  -- a tiny matvec, so the SE gate is
    known *before* the big second matmul and the blend fuses into PSUM
    evacuation.
  * matmuls run as float32r (full fp32 data, reduced-precision PE mode,
    1 cycle/row at free-dim >= 256 -- same speed as bf16).

Sharding: data-parallel over batch, 2 samples per core on 8 cores.
"""

import numpy as np

try:
    import concourse.bass as bass
except ImportError:
    import sys

    sys.path.insert(0, "/opt/trn_rl_repo")
    import concourse.bass as bass

import concourse.tile as tile
from concourse import bacc, mybir
from concourse.bass_utils import run_bass_kernel_spmd
from concourse.masks import make_identity

F32 = mybir.dt.float32
F32R = mybir.dt.float32r
AF = mybir.ActivationFunctionType
ALU = mybir.AluOpType
AX = mybir.AxisListType

B_TOTAL = 16
N_CORES = 8
B_PER_CORE = B_TOTAL // N_CORES  # 2
C = 512
N = 4096
CB = C // 128  # 4 c-blocks
KT = N // 128  # 32 n-slices for transpose/mm1
NCH = N // 512  # 8 n-chunks for mm2


def _build_program() -> bass.Bass:
    nc = bacc.Bacc(target_bir_lowering=False, debug=False)

    x_d = nc.dram_tensor("x", [B_PER_CORE, C, N], F32, kind="ExternalInput").ap()
    w1_d = nc.dram_tensor("w1", [64, 2 * C], F32, kind="ExternalInput").ap()
    b1_d = nc.dram_tensor("b1", [64, 1], F32, kind="ExternalInput").ap()
    w2_d = nc.dram_tensor("w2", [C, 64], F32, kind="ExternalInput").ap()
    b2_d = nc.dram_tensor("b2", [C, 1], F32, kind="ExternalInput").ap()
    y_d = nc.dram_tensor("y", [B_PER_CORE, C, N], F32, kind="ExternalOutput").ap()

    with tile.TileContext(nc) as tc:
        _emit(tc, x_d, w1_d, b1_d, w2_d, b2_d, y_d)
    nc.compile()
    return nc


def _emit(tc, x_d, w1_d, b1_d, w2_d, b2_d, y_d):
    nc = tc.nc
    from contextlib import ExitStack

    with ExitStack() as ctx:
        singles = ctx.enter_context(tc.tile_pool(name="singles", bufs=1))
        qpool = ctx.enter_context(tc.tile_pool(name="qpool", bufs=2))
        qtpool = ctx.enter_context(tc.tile_pool(name="qtpool", bufs=4))
        gpool = ctx.enter_context(tc.tile_pool(name="gpool", bufs=1))
        gtpool = ctx.enter_context(tc.tile_pool(name="gtpool", bufs=2))
        stats = ctx.enter_context(tc.tile_pool(name="stats", bufs=2))
        outp = ctx.enter_context(tc.tile_pool(name="outp", bufs=3))
        psum = ctx.enter_context(tc.tile_pool(name="psum", bufs=1, space="PSUM"))

        # sample-0 input stream first: these DMAs gate the whole pipeline
        q0 = qpool.tile([128, CB, N], F32R, tag="q", name="q_s0")
        for j in range(NCH):
            nsl = slice(512 * j, 512 * (j + 1))
            for m in range(CB):
                nc.sync.dma_start(
                    out=q0[:, m, nsl],
                    in_=x_d[0, 128 * m : 128 * (m + 1), nsl].bitcast(F32R),
                )

        # ---- one-time setup -------------------------------------------------
        ident = singles.tile([128, 128], F32)
        make_identity(nc, ident)
        ident_r = singles.tile([128, 128], F32R)
        nc.vector.tensor_copy(ident_r, ident)

        # w1T: [k=2C partitions over 8 tiles, m=64] packed as [128, 8*64]
        w1_nat = singles.tile([64, 2 * C], F32)
        nc.sync.dma_start(out=w1_nat, in_=w1_d)
        w1T = singles.tile([128, 8, 64], F32)
        for k in range(8):
            tp = psum.tile([128, 64], F32, tag="tstage", bufs=3)
            nc.tensor.transpose(
                tp, w1_nat[0:64, 128 * k : 128 * (k + 1)], ident[0:64, 0:64]
            )
            nc.vector.tensor_copy(w1T[:, k, :], tp)

        # w2T: [k=64, m=C over 4 tiles] packed as [64, 4, 128]
        w2_nat = singles.tile([128, CB, 64], F32)
        for m in range(CB):
            nc.sync.dma_start(
                out=w2_nat[:, m, :], in_=w2_d[128 * m : 128 * (m + 1), :]
            )
        w2T = singles.tile([64, CB, 128], F32)
        for m in range(CB):
            tp = psum.tile([128, 128], F32, tag="tstage", bufs=3)
            nc.tensor.transpose(tp[0:64, :], w2_nat[:, m, :], ident)
            nc.vector.tensor_copy(w2T[:, m, :], tp[0:64, :])

        b1_t = singles.tile([64, 1], F32)
        nc.sync.dma_start(out=b1_t, in_=b1_d)
        b2_t = singles.tile([128, CB], F32)
        for m in range(CB):
            nc.sync.dma_start(out=b2_t[:, m : m + 1], in_=b2_d[128 * m : 128 * (m + 1), :])

        # ---- per-sample pipeline -------------------------------------------
        for b in range(B_PER_CORE):
            # 1. q = x[b]: sample 0 was streamed in above; later samples load
            # here (chunked, double-buffered via the pool)
            if b == 0:
                q = q0
            else:
                q = qpool.tile([128, CB, N], F32R, tag="q", name=f"q_s{b}")
                for j in range(NCH):
                    nsl = slice(512 * j, 512 * (j + 1))
                    for m in range(CB):
                        nc.sync.dma_start(
                            out=q[:, m, nsl],
                            in_=x_d[b, 128 * m : 128 * (m + 1), nsl].bitcast(F32R),
                        )

            # 2. pooled_x on the otherwise-idle GpSimd: tensor_scalar
            # pass-through with accum_out gives per-partition row sums.
            px_mean = stats.tile([128, CB], F32, tag="px")
            px_part = stats.tile([128, CB, 2], F32, tag="pxp")
            for m in range(CB):
                for h in range(2):
                    hsl = slice(2048 * h, 2048 * (h + 1))
                    pxs = stats.tile([128, 2048], F32, tag="pxs", bufs=1)
                    nc.scalar.activation(
                        out=pxs,
                        in_=q[:, m, hsl].bitcast(F32),
                        func=AF.Copy,
                        accum_out=px_part[:, m, h : h + 1],
                    )
            px_raw = stats.tile([128, CB], F32, tag="pxr")
            nc.vector.tensor_reduce(out=px_raw, in_=px_part, axis=AX.X, op=ALU.add)
            nc.scalar.mul(px_mean, px_raw, 1.0 / N)

            # 3. energy = q @ q.T via on-the-fly PE transposes (fp32r matmul).
            # energy is symmetric: compute only the upper-triangular blocks
            # (row-block m covers cols >= 128m) and mirror the rest after.
            eps = [
                psum.tile([128, C - 128 * m], F32, tag="bank", bufs=5,
                          name=f"eps_{b}_{m}")
                for m in range(CB)
            ]
            for kt in range(KT):
                tps = psum.tile([128, C], F32, tag="tstage", bufs=3)
                sl = slice(128 * kt, 128 * (kt + 1))
                for m in range(CB):
                    nc.tensor.transpose(
                        tps[:, 128 * m : 128 * (m + 1)].bitcast(F32R),
                        q[:, m, sl],
                        ident_r,
                    )
                qt = qtpool.tile([128, C], F32R, tag="qt")
                if kt % 2 == 0:
                    nc.vector.tensor_copy(qt, tps)
                else:
                    nc.scalar.copy(qt, tps)
                for m in range(CB):
                    nc.tensor.matmul(
                        eps[m],
                        lhsT=qt[:, 128 * m : 128 * (m + 1)],
                        rhs=qt[:, 128 * m :],
                        start=(kt == 0),
                        stop=(kt == KT - 1),
                    )

            # 3b. evacuate energy to SBUF (upper blocks), mirror lower blocks
            # via PE transpose of the upper ones.
            en = gpool.tile([128, CB, C], F32, tag="en")
            for m in range(CB):
                (nc.scalar.copy if m % 2 else nc.vector.tensor_copy)(
                    en[:, m, 128 * m :], eps[m]
                )
            for m in range(1, CB):
                tps = psum.tile([128, C], F32, tag="tstage", bufs=3)
                for j in range(m):
                    # block (m, j) = block (j, m)^T
                    nc.tensor.transpose(
                        tps[:, 128 * j : 128 * (j + 1)],
                        en[:, j, 128 * m : 128 * (m + 1)],
                        ident,
                    )
                (nc.scalar.copy if m % 2 else nc.vector.tensor_copy)(
                    en[:, m, : 128 * m], tps[:, : 128 * m]
                )

            # 4+5 fused: per-block softmax stats feed PE transposes as
            # they land.  GT staging uses the "bank" slots freed by eps.
            nmin = stats.tile([128, CB], F32, tag="nmin")
            G = gpool.tile([128, CB, C], F32, tag="G")
            S = stats.tile([128, CB], F32, tag="S")
            gstage = [
                psum.tile([128, C], F32, tag="bank", bufs=5, name=f"gst_{b}_{k}")
                for k in range(CB)
            ]
            for m in range(CB):
                nc.vector.tensor_reduce(
                    out=nmin[:, m : m + 1], in_=en[:, m, :], axis=AX.X, op=ALU.min
                )
                nc.scalar.activation(
                    out=G[:, m, :],
                    in_=en[:, m, :],
                    func=AF.Exp,
                    bias=nmin[:, m : m + 1],
                    scale=-1.0,
                    accum_out=S[:, m : m + 1],
                )
                for k in range(CB):
                    nc.tensor.transpose(
                        gstage[k][:, 128 * m : 128 * (m + 1)],
                        G[:, m, 128 * k : 128 * (k + 1)],
                        ident,
                    )
            recipS = stats.tile([128, CB], F32, tag="rS")
            nc.vector.reciprocal(recipS, S)
            GT = gtpool.tile([128, CB, C], F32R, tag="GT")
            for k in range(CB):
                nc.scalar.copy(GT[:, k, : C // 2], gstage[k][:, : C // 2])
                nc.vector.tensor_copy(GT[:, k, C // 2 :], gstage[k][:, C // 2 :])

            # 6. pooled_out = (G @ px_mean) / S
            ps_po = psum.tile([128, CB], F32, tag="tstage", bufs=3)
            for m in range(CB):
                for k in range(CB):
                    nc.tensor.matmul(
                        ps_po[:, m : m + 1],
                        lhsT=GT[:, k, 128 * m : 128 * (m + 1)].bitcast(F32),
                        rhs=px_mean[:, k : k + 1],
                        start=(k == 0),
                        stop=(k == CB - 1),
                    )
            po_mean = stats.tile([128, CB], F32, tag="po")
            nc.vector.tensor_mul(po_mean, ps_po, recipS)

            # 7. SE gate: h = relu(w1@pooled+b1); se = sigmoid(w2@h+b2)
            ps_h = psum.tile([64, 1], F32, tag="tstage", bufs=3)
            for k in range(8):
                rhs = px_mean[:, k : k + 1] if k < 4 else po_mean[:, k - 4 : k - 3]
                nc.tensor.matmul(
                    ps_h,
                    lhsT=w1T[:, k, :],
                    rhs=rhs,
                    start=(k == 0),
                    stop=(k == 7),
                )
            h_sb = stats.tile([64, 1], F32, tag="h")
            nc.scalar.activation(h_sb, ps_h, AF.Relu, bias=b1_t)

            ps_se = psum.tile([128, CB], F32, tag="tstage", bufs=3)
            for m in range(CB):
                nc.tensor.matmul(
                    ps_se[:, m : m + 1],
                    lhsT=w2T[:, m, :],
                    rhs=h_sb,
                    start=True,
                    stop=True,
                )
            se = stats.tile([128, CB], F32, tag="se")
            for m in range(CB):
                nc.scalar.activation(
                    se[:, m : m + 1], ps_se[:, m : m + 1], AF.Sigmoid,
                    bias=b2_t[:, m : m + 1],
                )
            beta0 = stats.tile([128, CB], F32, tag="b0")
            beta = stats.tile([128, CB], F32, tag="b1")
            nc.vector.tensor_scalar(
                out=beta0, in0=se, scalar1=-1.0, scalar2=1.0, op0=ALU.mult, op1=ALU.add
            )
            nc.vector.tensor_mul(beta, beta0, recipS)

            # 8. out_raw = G @ q per chunk; fused blend y = se*x + beta*out_raw
            for j in range(NCH):
                nsl = slice(512 * j, 512 * (j + 1))
                for m in range(CB):
                    if m == 3:
                        ps_o = psum.tile([128, 512], F32, tag="tstage", bufs=3)
                    else:
                        ps_o = psum.tile([128, 512], F32, tag="bank", bufs=5)
                    for k in range(CB):
                        nc.tensor.matmul(
                            ps_o,
                            lhsT=GT[:, k, 128 * m : 128 * (m + 1)],
                            rhs=q[:, k, nsl],
                            start=(k == 0),
                            stop=(k == CB - 1),
                        )
                    ob = outp.tile([128, 512], F32, tag="ob", bufs=6)
                    nc.scalar.activation(
                        ob, ps_o, AF.Copy, scale=beta[:, m : m + 1]
                    )
                    fin = outp.tile([128, 512], F32, tag="fin", bufs=4)
                    nc.vector.scalar_tensor_tensor(
                        out=fin,
                        in0=q[:, m, nsl].bitcast(F32),
                        scalar=se[:, m : m + 1],
                        in1=ob,
                        op0=ALU.mult,
                        op1=ALU.add,
                    )
                    nc.sync.dma_start(
                        out=y_d[b, 128 * m : 128 * (m + 1), nsl], in_=fin
                    )


_NC_CACHE = None


def _get_program():
    global _NC_CACHE
    if _NC_CACHE is None:
        _NC_CACHE = _build_program()
    return _NC_CACHE


def kernel(x, w1, b1, w2, b2, _trace=False):
    x = np.ascontiguousarray(x, dtype=np.float32)
    B, Cc, H, W = x.shape
    assert (B, Cc, H * W) == (B_TOTAL, C, N)
    xr = x.reshape(B, Cc, H * W)
    in_maps = []
    for i in range(N_CORES):
        in_maps.append(
            {
                "x": np.ascontiguousarray(xr[B_PER_CORE * i : B_PER_CORE * (i + 1)]),
                "w1": np.ascontiguousarray(w1, dtype=np.float32),
                "b1": np.ascontiguousarray(b1, dtype=np.float32).reshape(64, 1),
                "w2": np.ascontiguousarray(w2, dtype=np.float32),
                "b2": np.ascontiguousarray(b2, dtype=np.float32).reshape(C, 1),
            }
        )
    nc = _get_program()
    res = run_bass_kernel_spmd(nc, in_maps, list(range(N_CORES)), trace=_trace)
    y = np.concatenate([res.results[i]["y"] for i in range(N_CORES)], axis=0)
    out = y.reshape(B, Cc, H, W).astype(np.float32)
    if _trace:
        return out, res
    return out
